# revision 1
# baseline (speedup 1.0000x reference)
"""Trainium2 Bass kernel for a 2-layer GCN (nn_EvenLamerGCN).

reference semantics (PyG GCNConv x2, eval mode):
    deg[i]  = 1 + indeg(i)                (self-loops added)
    dinv    = deg ** -0.5
    h  = relu(A_hat @ (x @ W1) + b1),  A_hat = D^-1/2 (A + I) D^-1/2
    o  = A_hat @ (h @ W2) + b2
    return o, log_softmax(o, axis=1)

Distribution: nodes sharded over 8 NeuronCores (12500/core, padded to
12544), edges partitioned by destination core.  The per-edge norm is
folded into per-node row scalings:
    out = dinv * ( sum_{e: dst=i} T[src_e] + T[i] ),   T = dinv * (x @ W)

Per layer on each core:
  1. dense matmul -> row-scaled table shard T_c, AllGather -> full T
  2. per-edge dma_gather of T[src] rows (128-row chunks, int16 idx against
     static table windows)
  3. segment-sum via one-hot matmul: S^T[e, d] = (dst_local[e] % 128 == d)
     built on DVE from shipped per-slot ids, then PSUM-accumulated
     out_block += S^T.T @ gathered, drained into a [128, 98, d] SBUF
     accumulator seeded with the self-loop term.
Edges are laid out per (dst-block, src-window) cell with a uniform
chunk quota so the instruction stream is identical on all 8 cores
(SPMD, one NEFF); all per-core variation lives in input data.
"""

import sys

for _p in ("/opt/trn_rl_repo", "/root/.axon_site/_ro/trn_rl_repo"):
    if _p not in sys.path:
        sys.path.insert(0, _p)

from contextlib import ExitStack
from dataclasses import dataclass

import numpy as np

import concourse.bass as bass
import concourse.mybir as mybir
import concourse.tile as tile
from concourse import bacc
from concourse.bass import ds, ts
from concourse.bass_utils import run_bass_kernel_spmd
from concourse.masks import make_identity

F32 = mybir.dt.float32
BF16 = mybir.dt.bfloat16
I16 = mybir.dt.int16
AF = mybir.ActivationFunctionType
ALU = mybir.AluOpType


@dataclass(frozen=True)
class Cfg:
    n: int = 100000          # nodes
    din: int = 512           # input features
    dh: int = 128            # hidden features
    dout: int = 40           # output features
    cores: int = 8
    wsize: int = 32768       # int16 gather window (rows)
    max_piece: int = 32      # chunks per gather instruction

    @property
    def nsh(self):           # real nodes per core
        return self.n // self.cores

    @property
    def nloc(self):          # padded nodes per core (multiple of 128)
        return ((self.nsh + 127) // 128) * 128

    @property
    def nt(self):            # 128-node dst blocks per core
        return self.nloc // 128

    @property
    def trows(self):         # rows in the gathered tables
        return self.cores * self.nloc

    @property
    def dh2(self):           # padded output width
        return max(64, ((self.dout + 63) // 64) * 64)

    @property
    def dt2(self):           # layer-2 bf16 table row width (256B rows)
        return max(128, self.dh2)

    @property
    def kt(self):            # k-tiles in the first matmul
        return self.din // 128

    @property
    def nwin(self):          # number of static src windows
        return max(1, -(-self.trows // self.wsize))

    @property
    def wbases(self):
        return [min(w * self.wsize, self.trows - self.wsize)
                for w in range(self.nwin)]


@dataclass(frozen=True)
class Plan:
    quotas: tuple          # chunks per (window) cell, per dst block
    sections: tuple        # per window: list of piece sizes (in chunks)

    @property
    def chunks_per_block(self):
        return sum(self.quotas)

    @property
    def total_chunks(self):
        return sum(sum(s) for s in self.sections)


# ----------------------------------------------------------------------------
# CPU-side preprocessing
# ----------------------------------------------------------------------------

def preprocess(cfg: Cfg, edge_index: np.ndarray):
    c = cfg
    src = np.asarray(edge_index[0], dtype=np.int64)
    dst = np.asarray(edge_index[1], dtype=np.int64)

    deg = np.bincount(dst, minlength=c.n).astype(np.float32) + 1.0
    deg_pt = np.ones((c.cores, 128, c.nt), np.float32)
    for ci in range(c.cores):
        dl = np.ones(c.nloc, np.float32)
        dl[: c.nsh] = deg[ci * c.nsh : (ci + 1) * c.nsh]
        deg_pt[ci] = dl.reshape(c.nt, 128).T

    row_of = lambda i: (i // c.nsh) * c.nloc + (i % c.nsh)
    r_all = row_of(src)
    w_all = np.minimum(r_all // c.wsize, c.nwin - 1)
    core_all = dst // c.nsh
    dloc_all = dst - core_all * c.nsh
    b_all = dloc_all // 128
    id_all = dloc_all % 128

    # count edges per (core, block, window) -> uniform chunk quotas
    cell_key = (core_all * c.nt + b_all) * c.nwin + w_all
    counts = np.bincount(cell_key, minlength=c.cores * c.nt * c.nwin)
    counts = counts.reshape(c.cores, c.nt, c.nwin)
    quotas = tuple(int(-(-counts[:, :, w].max() // 128)) for w in range(c.nwin))

    # piece sizes (chunks) per window section
    sections = []
    for w in range(c.nwin):
        sec = c.nt * quotas[w]
        sizes = []
        while sec > 0:
            sizes.append(min(c.max_piece, sec))
            sec -= sizes[-1]
        sections.append(tuple(sizes))
    plan = Plan(quotas=quotas, sections=tuple(sections))

    total_chunks = plan.total_chunks
    slots = total_chunks * 128
    bases = c.wbases

    idx16 = np.zeros((c.cores, 128, slots // 16), np.int16)
    ids_f32 = np.empty((c.cores, 128, total_chunks), np.float32)

    order = np.lexsort((r_all, w_all, b_all, core_all))
    so_r, so_w, so_b, so_core, so_id = (
        r_all[order], w_all[order], b_all[order], core_all[order], id_all[order]
    )
    core_starts = np.searchsorted(so_core, np.arange(c.cores + 1))

    for ci in range(c.cores):
        lo, hi = core_starts[ci], core_starts[ci + 1]
        rr, ii = so_r[lo:hi], so_id[lo:hi]
        rel = np.zeros(slots, np.int64)      # window-relative gather rows
        ids = np.full(slots, -1.0, np.float32)
        # slot offset of window section w
        sec_off = np.cumsum([0] + [c.nt * q * 128 for q in quotas])
        pos = 0
        # sorted order within a core is (b, w, r); cells land at
        # sec_off[w] + b * quotas[w] * 128
        for b in range(c.nt):
            for w in range(c.nwin):
                cnt = counts[ci, b, w]
                if cnt:
                    off = sec_off[w] + b * quotas[w] * 128
                    rel[off : off + cnt] = rr[pos : pos + cnt] - bases[w]
                    ids[off : off + cnt] = ii[pos : pos + cnt]
                    pos += cnt
        assert pos == hi - lo
        assert rel.min() >= 0 and rel.max() < c.wsize

        v = rel.reshape(-1, 16)              # slot i at [i%16, i//16]
        wrapped = np.ascontiguousarray(v.T)  # [16, slots/16]
        idx16[ci] = np.tile(wrapped, (8, 1)).astype(np.int16)
        ids_f32[ci] = ids.reshape(total_chunks, 128).T

    return deg_pt, idx16, ids_f32, plan


# ----------------------------------------------------------------------------
# Device kernel
# ----------------------------------------------------------------------------

def build(nc, tc, cfg: Cfg, plan: Plan):
    c = cfg
    RG = [list(range(c.cores))]
    total_chunks = plan.total_chunks
    slots = total_chunks * 128

    x_sh = nc.dram_tensor("x_sh", [c.nloc, c.din], BF16, kind="ExternalInput").ap()
    w1 = nc.dram_tensor("w1", [c.din, c.dh], BF16, kind="ExternalInput").ap()
    w2 = nc.dram_tensor("w2", [c.dh, c.dh2], F32, kind="ExternalInput").ap()
    b1r = nc.dram_tensor("b1r", [128, c.dh], F32, kind="ExternalInput").ap()
    b2r = nc.dram_tensor("b2r", [128, c.dh2], F32, kind="ExternalInput").ap()
    degp = nc.dram_tensor("degp", [128, c.nt], F32, kind="ExternalInput").ap()
    idx16 = nc.dram_tensor("idx16", [128, slots // 16], I16, kind="ExternalInput").ap()
    idsf = nc.dram_tensor("idsf", [128, total_chunks], BF16, kind="ExternalInput").ap()
    out_h = nc.dram_tensor("out_h", [c.nloc, c.dh2], F32, kind="ExternalOutput").ap()
    out_ls = nc.dram_tensor("out_ls", [c.nloc, c.dh2], F32, kind="ExternalOutput").ap()

    t1_loc = nc.dram_tensor("t1_loc", [c.nloc, c.dh], BF16, kind="Internal").ap()
    t1_full = nc.dram_tensor(
        "t1_full", [c.trows, c.dh], BF16, kind="Internal", addr_space="Shared"
    ).ap()
    t2_loc = nc.dram_tensor("t2_loc", [c.nloc, c.dt2], BF16, kind="Internal").ap()
    t2_full = nc.dram_tensor(
        "t2_full", [c.trows, c.dt2], BF16, kind="Internal", addr_space="Shared"
    ).ap()

    with ExitStack() as st:
        cpool = st.enter_context(tc.tile_pool(name="consts", bufs=1))
        accp = st.enter_context(tc.tile_pool(name="acc", bufs=1))
        gp = st.enter_context(tc.tile_pool(name="gp", bufs=2))
        sp = st.enter_context(tc.tile_pool(name="sp", bufs=2))
        pp = st.enter_context(tc.tile_pool(name="pp", bufs=2))
        ppsum = st.enter_context(tc.tile_pool(name="ppsum", bufs=4, space="PSUM"))
        p0 = st.enter_context(tc.tile_pool(name="p0", bufs=3))
        p0ps = st.enter_context(tc.tile_pool(name="p0ps", bufs=2, space="PSUM"))
        p0psT = st.enter_context(tc.tile_pool(name="p0psT", bufs=2, space="PSUM"))

        # ---- constants ----
        ident = cpool.tile([128, 128], F32)
        make_identity(nc, ident)
        identb = cpool.tile([128, 128], BF16)
        make_identity(nc, identb)
        w1sb = cpool.tile([128, c.kt, c.dh], BF16)
        nc.sync.dma_start(w1sb, w1.rearrange("(o p) f -> p o f", p=128))
        w2sb = cpool.tile([128, c.dh2], F32)
        nc.sync.dma_start(w2sb, w2)
        b1sb = cpool.tile([128, c.dh], F32)
        nc.sync.dma_start(b1sb, b1r)
        b2sb = cpool.tile([128, c.dh2], F32)
        nc.sync.dma_start(b2sb, b2r)
        dinv = cpool.tile([128, c.nt], F32)
        nc.sync.dma_start(dinv, degp)
        nc.scalar.activation(dinv, dinv, AF.Sqrt)
        nc.vector.reciprocal(dinv, dinv)
        iota = cpool.tile([128, c.max_piece, 128], BF16)
        nc.gpsimd.iota(iota, pattern=[[0, c.max_piece], [1, 128]], base=0,
                       channel_multiplier=0,
                       allow_small_or_imprecise_dtypes=True)

        # ---- phase 0: T1 = dinv * (x @ W1), write local table shard ----
        for t in range(c.nt):
            xt = p0.tile([128, c.din], BF16, tag="xt")
            nc.sync.dma_start(xt, x_sh[ts(t, 128), :])
            hps = p0ps.tile([128, c.dh], F32, tag="hps")
            for j in range(c.kt):
                tps = p0psT.tile([128, 128], BF16, tag="tps")
                nc.tensor.transpose(tps, xt[:, ts(j, 128)], identb)
                xT = p0.tile([128, 128], BF16, tag="xT")
                nc.vector.tensor_copy(xT, tps)
                nc.tensor.matmul(
                    hps, lhsT=xT, rhs=w1sb[:, j, :],
                    start=(j == 0), stop=(j == c.kt - 1),
                )
            hsb = p0.tile([128, c.dh], BF16, tag="hsb")
            nc.vector.tensor_scalar_mul(hsb, hps, dinv[:, t : t + 1])
            nc.sync.dma_start(t1_loc[ts(t, 128), :], hsb)

        nc.gpsimd.collective_compute(
            "AllGather", ALU.bypass, replica_groups=RG,
            ins=[t1_loc.opt()], outs=[t1_full.opt()],
        )

        # ---- edge aggregation: acc[:, b, :] += sum over block's chunks ----
        def edge_phase(table_full, t_loc, acc, d, dt):
            # seed with the self-loop term T[i] (sync DMA + DVE cast; keeps
            # the SWDGE lanes exclusively on queue-3 gathers)
            tv = t_loc.rearrange("(b p) f -> p b f", p=128)
            for t in range(c.nt):
                sd = p0.tile([128, dt], BF16, tag="sd")
                nc.sync.dma_start(sd, tv[:, t, :])
                nc.vector.tensor_copy(acc[:, t, :], sd[:, :d])
            bases = c.wbases
            max_sec = max(sum(sz) for sz in plan.sections)
            chunk0 = 0          # global chunk cursor
            for w, sizes in enumerate(plan.sections):
                q = plan.quotas[w]
                sec_ch = sum(sizes)
                sit = sp.tile([128, max_sec * 8], I16, tag="sit")
                nc.sync.dma_start(
                    sit[:, : sec_ch * 8],
                    idx16[:, chunk0 * 8 : (chunk0 + sec_ch) * 8],
                )
                sid = sp.tile([128, max_sec], BF16, tag="sid")
                nc.sync.dma_start(sid[:, :sec_ch], idsf[:, chunk0 : chunk0 + sec_ch])
                loc = 0
                k_in_block = 0
                b = 0
                ps = None
                for nch in sizes:
                    g = gp.tile([128, c.max_piece, dt], BF16, tag="gt")
                    nc.gpsimd.dma_gather(
                        g[:, :nch, :], table_full[ds(bases[w], c.wsize), :],
                        sit[:, loc * 8 : (loc + nch) * 8],
                        num_idxs=nch * 128, num_idxs_reg=nch * 128, elem_size=dt,
                        single_packet=False, queue_num=3,
                    )
                    stt = pp.tile([128, c.max_piece, 128], BF16, tag="stt")
                    nc.vector.tensor_tensor(
                        stt[:, :nch, :], iota[:, :nch, :],
                        sid[:, loc : loc + nch, None].to_broadcast((128, nch, 128)),
                        ALU.is_equal,
                    )
                    for j in range(nch):
                        if k_in_block == 0:
                            ps = ppsum.tile([128, d], F32, tag="ps")
                        nc.tensor.matmul(
                            ps, lhsT=stt[:, j, :], rhs=g[:, j, :d],
                            start=(k_in_block == 0), stop=(k_in_block == q - 1),
                        )
                        k_in_block += 1
                        if k_in_block == q:
                            nc.vector.tensor_tensor(
                                acc[:, b, :], acc[:, b, :], ps, ALU.add
                            )
                            b += 1
                            k_in_block = 0
                    loc += nch
                    chunk0 += nch
                assert b == c.nt and k_in_block == 0

        acc1 = accp.tile([128, c.nt, c.dh], F32)
        edge_phase(t1_full, t1_loc, acc1, c.dh, c.dh)

        # ---- g1 = dinv * relu(dinv * agg + b1), in place, batched ----
        dinv_bc1 = dinv[:, :, None].to_broadcast((128, c.nt, c.dh))
        nc.vector.tensor_tensor(acc1, acc1, dinv_bc1, ALU.mult)
        nc.vector.tensor_tensor(
            acc1, acc1, b1sb[:, None, :].to_broadcast((128, c.nt, c.dh)), ALU.add
        )
        nc.scalar.activation(acc1, acc1, AF.Relu)
        nc.vector.tensor_tensor(acc1, acc1, dinv_bc1, ALU.mult)

        # ---- phase 2: T2 = g1 @ W2 (row scaling already folded into g1) ----
        for t in range(c.nt):
            tps = p0psT.tile([128, 128], F32, tag="tps")
            nc.tensor.transpose(tps, acc1[:, t, :], ident)
            gT = p0.tile([128, 128], F32, tag="xT")
            nc.vector.tensor_copy(gT, tps)
            h2ps = p0ps.tile([128, c.dh2], F32, tag="hps")
            nc.tensor.matmul(h2ps, lhsT=gT, rhs=w2sb, start=True, stop=True)
            h2sb = p0.tile([128, c.dh2], BF16, tag="h2sb")
            nc.vector.tensor_copy(h2sb, h2ps)
            nc.sync.dma_start(t2_loc[ts(t, 128), : c.dh2], h2sb)

        nc.gpsimd.collective_compute(
            "AllGather", ALU.bypass, replica_groups=RG,
            ins=[t2_loc.opt()], outs=[t2_full.opt()],
        )

        # ---- layer-2 edge aggregation ----
        acc2 = accp.tile([128, c.nt, c.dh2], F32)
        edge_phase(t2_full, t2_loc, acc2, c.dh2, c.dt2)

        # ---- h = dinv * agg2 + b2 ; log_softmax (batched) ----
        ohv = out_h.rearrange("(t p) f -> p t f", p=128)
        olv = out_ls.rearrange("(t p) f -> p t f", p=128)
        nc.vector.tensor_tensor(
            acc2, acc2, dinv[:, :, None].to_broadcast((128, c.nt, c.dh2)), ALU.mult
        )
        nc.vector.tensor_tensor(
            acc2, acc2, b2sb[:, None, :].to_broadcast((128, c.nt, c.dh2)), ALU.add
        )
        nc.sync.dma_start(ohv, acc2)
        accN = acc2[:, :, : c.dout]
        mx = accp.tile([128, c.nt], F32, tag="mx")
        nc.vector.tensor_reduce(mx, accN, mybir.AxisListType.X, ALU.max)
        nc.vector.tensor_tensor(
            accN, accN, mx[:, :, None].to_broadcast((128, c.nt, c.dout)), ALU.subtract
        )
        e1 = accp.tile([128, c.nt, c.dout], F32, tag="e1")
        nc.scalar.activation(e1, accN, AF.Exp)
        se = accp.tile([128, c.nt], F32, tag="se")
        nc.vector.tensor_reduce(se, e1, mybir.AxisListType.X, ALU.add)
        ln = accp.tile([128, c.nt], F32, tag="ln")
        nc.scalar.activation(ln, se, AF.Ln)
        nc.vector.tensor_tensor(
            accN, accN, ln[:, :, None].to_broadcast((128, c.nt, c.dout)), ALU.subtract
        )
        nc.sync.dma_start(olv[:, :, : c.dout], accN)


# ----------------------------------------------------------------------------
# Host entry point
# ----------------------------------------------------------------------------

_CACHE = {}


def _get_compiled(cfg: Cfg, plan: Plan):
    key = (cfg, plan)
    if key not in _CACHE:
        nc = bacc.Bacc(
            "TRN2", target_bir_lowering=False, debug=False,
            num_devices=cfg.cores, num_swdge_queues=4,
        )
        with tile.TileContext(nc) as tc:
            build(nc, tc, cfg, plan)
        nc.compile()
        _CACHE[key] = nc
    return _CACHE[key]


def make_in_maps(cfg: Cfg, x, W1, b1, W2, b2, deg_pt, idx16, ids_f32):
    import ml_dtypes

    c = cfg
    x = np.asarray(x, np.float32)
    w2p = np.zeros((c.dh, c.dh2), np.float32)
    w2p[:, : c.dout] = np.asarray(W2, np.float32)
    b1rep = np.tile(np.asarray(b1, np.float32)[None, :], (128, 1))
    b2p = np.zeros(c.dh2, np.float32)
    b2p[: c.dout] = np.asarray(b2, np.float32)
    b2rep = np.tile(b2p[None, :], (128, 1))
    w1c = np.ascontiguousarray(
        np.asarray(W1, np.float32).astype(ml_dtypes.bfloat16)
    )

    in_maps = []
    for ci in range(c.cores):
        xs = np.zeros((c.nloc, c.din), ml_dtypes.bfloat16)
        xs[: c.nsh] = x[ci * c.nsh : (ci + 1) * c.nsh].astype(ml_dtypes.bfloat16)
        in_maps.append({
            "x_sh": xs,
            "w1": w1c,
            "w2": w2p,
            "b1r": b1rep,
            "b2r": b2rep,
            "degp": np.ascontiguousarray(deg_pt[ci]),
            "idx16": np.ascontiguousarray(idx16[ci]),
            "idsf": np.ascontiguousarray(ids_f32[ci].astype(ml_dtypes.bfloat16)),
        })
    return in_maps


def _ensure_ntff_hook():
    """Install the axon NTFF profile hook if the image's antenv lacks it."""
    import types

    try:
        from antenv.axon_hooks import get_axon_ntff_profile_hook  # noqa: F401
        return
    except ImportError:
        pass
    import antenv

    m = types.ModuleType("antenv.axon_hooks")
    m._hook = None
    m.set_axon_ntff_profile_hook = lambda h: setattr(m, "_hook", h)
    m.get_axon_ntff_profile_hook = lambda: m._hook
    sys.modules["antenv.axon_hooks"] = m
    antenv.axon_hooks = m
    try:
        from trn_agent_boot.trn_boot import _ntff_profile_via_ctypes

        h = _ntff_profile_via_ctypes("/opt/axon/libaxon_pjrt.so")
        if h is not None:
            m._hook = h
    except Exception as e:
        print(f"ntff hook install failed: {e}")

    from concourse import bass_utils as bu

    bu.upload_artifacts = lambda tmpdir: tmpdir


def run(cfg: Cfg, inputs: dict, trace: bool = False):
    if trace:
        _ensure_ntff_hook()
    deg_pt, idx16, ids_f32, plan = preprocess(cfg, inputs["edge_index"])
    nc = _get_compiled(cfg, plan)
    in_maps = make_in_maps(
        cfg, inputs["x"], inputs["W1"], inputs["b1"], inputs["W2"], inputs["b2"],
        deg_pt, idx16, ids_f32,
    )
    res = run_bass_kernel_spmd(
        nc, in_maps, core_ids=list(range(cfg.cores)), trace=trace
    )
    c = cfg
    h = np.concatenate(
        [res.results[ci]["out_h"][: c.nsh, : c.dout] for ci in range(c.cores)], axis=0
    )
    ls = np.concatenate(
        [res.results[ci]["out_ls"][: c.nsh, : c.dout] for ci in range(c.cores)], axis=0
    )
    return (h, ls), res


def kernel(**inputs):
    (h, ls), _ = run(Cfg(), inputs)
    return h, ls



# revision 8
# speedup vs baseline: 1.1537x; 1.1537x over previous
"""Trainium2 Bass kernel for a 2-layer GCN (nn_EvenLamerGCN).

reference semantics (PyG GCNConv x2, eval mode):
    deg[i]  = 1 + indeg(i)                (self-loops added)
    dinv    = deg ** -0.5
    h  = relu(A_hat @ (x @ W1) + b1),  A_hat = D^-1/2 (A + I) D^-1/2
    o  = A_hat @ (h @ W2) + b2
    return o, log_softmax(o, axis=1)

Distribution: nodes sharded over 8 NeuronCores (12500/core, padded to
12544), edges partitioned by destination core.  The per-edge norm is
folded into per-node row scalings:
    out = dinv * ( sum_{e: dst=i} T[src_e] + T[i] ),   T = dinv * (x @ W)

Per layer on each core:
  1. dense matmul -> row-scaled table shard T_c, AllGather -> full T
  2. per-edge dma_gather of T[src] rows (128-row chunks, int16 idx against
     static table windows)
  3. segment-sum via one-hot matmul: S^T[e, d] = (dst_local[e] % 128 == d)
     built on DVE from shipped per-slot ids, then PSUM-accumulated
     out_block += S^T.T @ gathered, drained into a [128, 98, d] SBUF
     accumulator seeded with the self-loop term.
Edges are laid out per (dst-block, src-window) cell with a uniform
chunk quota so the instruction stream is identical on all 8 cores
(SPMD, one NEFF); all per-core variation lives in input data.
"""

import sys

for _p in ("/opt/trn_rl_repo", "/root/.axon_site/_ro/trn_rl_repo"):
    if _p not in sys.path:
        sys.path.insert(0, _p)

from contextlib import ExitStack
from dataclasses import dataclass

import numpy as np

import concourse.bass as bass
import concourse.mybir as mybir
import concourse.tile as tile
from concourse import bacc
from concourse.bass import ds, ts
from concourse.bass_utils import run_bass_kernel_spmd
from concourse.masks import make_identity

F32 = mybir.dt.float32
BF16 = mybir.dt.bfloat16
I16 = mybir.dt.int16
AF = mybir.ActivationFunctionType
ALU = mybir.AluOpType


@dataclass(frozen=True)
class Cfg:
    n: int = 100000          # nodes
    din: int = 512           # input features
    dh: int = 128            # hidden features
    dout: int = 40           # output features
    cores: int = 8
    wsize: int = 32768       # int16 gather window (rows)
    max_piece: int = 32      # chunks per gather instruction

    @property
    def nsh(self):           # real nodes per core
        return self.n // self.cores

    @property
    def nloc(self):          # padded nodes per core (multiple of 128)
        return ((self.nsh + 127) // 128) * 128

    @property
    def nt(self):            # 128-node dst blocks per core
        return self.nloc // 128

    @property
    def trows(self):         # rows in the gathered tables
        return self.cores * self.nloc

    @property
    def dh2(self):           # padded output width
        return max(64, ((self.dout + 63) // 64) * 64)

    @property
    def dt2(self):           # layer-2 bf16 table row width (256B rows)
        return max(128, self.dh2)

    @property
    def kt(self):            # k-tiles in the first matmul
        return self.din // 128

    @property
    def nwin(self):          # number of static src windows
        return max(1, -(-self.trows // self.wsize))

    @property
    def wbases(self):
        return [min(w * self.wsize, self.trows - self.wsize)
                for w in range(self.nwin)]


@dataclass(frozen=True)
class Plan:
    quotas: tuple          # chunks per (window) cell, per dst block
    sections: tuple        # per window: list of piece sizes (in chunks)

    @property
    def chunks_per_block(self):
        return sum(self.quotas)

    @property
    def total_chunks(self):
        return sum(sum(s) for s in self.sections)


# ----------------------------------------------------------------------------
# CPU-side preprocessing
# ----------------------------------------------------------------------------

def preprocess(cfg: Cfg, edge_index: np.ndarray):
    c = cfg
    src = np.asarray(edge_index[0], dtype=np.int64)
    dst = np.asarray(edge_index[1], dtype=np.int64)

    deg = np.bincount(dst, minlength=c.n).astype(np.float32) + 1.0
    deg_pt = np.ones((c.cores, 128, c.nt), np.float32)
    for ci in range(c.cores):
        dl = np.ones(c.nloc, np.float32)
        dl[: c.nsh] = deg[ci * c.nsh : (ci + 1) * c.nsh]
        deg_pt[ci] = dl.reshape(c.nt, 128).T

    row_of = lambda i: (i // c.nsh) * c.nloc + (i % c.nsh)
    r_all = row_of(src)
    w_all = np.minimum(r_all // c.wsize, c.nwin - 1)
    core_all = dst // c.nsh
    dloc_all = dst - core_all * c.nsh
    b_all = dloc_all // 128
    id_all = dloc_all % 128

    # count edges per (core, block, window) -> uniform chunk quotas
    cell_key = (core_all * c.nt + b_all) * c.nwin + w_all
    counts = np.bincount(cell_key, minlength=c.cores * c.nt * c.nwin)
    counts = counts.reshape(c.cores, c.nt, c.nwin)
    quotas = tuple(int(-(-counts[:, :, w].max() // 128)) for w in range(c.nwin))

    # piece sizes (chunks) per window section
    sections = []
    for w in range(c.nwin):
        sec = c.nt * quotas[w]
        sizes = []
        while sec > 0:
            sizes.append(min(c.max_piece, sec))
            sec -= sizes[-1]
        sections.append(tuple(sizes))
    plan = Plan(quotas=quotas, sections=tuple(sections))

    total_chunks = plan.total_chunks
    slots = total_chunks * 128
    bases = c.wbases

    idx16 = np.zeros((c.cores, 128, slots // 16), np.int16)
    ids_f32 = np.empty((c.cores, 128, total_chunks), np.float32)

    order = np.lexsort((r_all, w_all, b_all, core_all))
    so_r, so_w, so_b, so_core, so_id = (
        r_all[order], w_all[order], b_all[order], core_all[order], id_all[order]
    )
    core_starts = np.searchsorted(so_core, np.arange(c.cores + 1))

    for ci in range(c.cores):
        lo, hi = core_starts[ci], core_starts[ci + 1]
        rr, ii = so_r[lo:hi], so_id[lo:hi]
        rel = np.zeros(slots, np.int64)      # window-relative gather rows
        ids = np.full(slots, -1.0, np.float32)
        # slot offset of window section w
        sec_off = np.cumsum([0] + [c.nt * q * 128 for q in quotas])
        pos = 0
        # sorted order within a core is (b, w, r); cells land at
        # sec_off[w] + b * quotas[w] * 128
        for b in range(c.nt):
            for w in range(c.nwin):
                cnt = counts[ci, b, w]
                if cnt:
                    off = sec_off[w] + b * quotas[w] * 128
                    rel[off : off + cnt] = rr[pos : pos + cnt] - bases[w]
                    ids[off : off + cnt] = ii[pos : pos + cnt]
                    pos += cnt
        assert pos == hi - lo
        assert rel.min() >= 0 and rel.max() < c.wsize

        v = rel.reshape(-1, 16)              # slot i at [i%16, i//16]
        wrapped = np.ascontiguousarray(v.T)  # [16, slots/16]
        idx16[ci] = np.tile(wrapped, (8, 1)).astype(np.int16)
        ids_f32[ci] = ids.reshape(total_chunks, 128).T

    return deg_pt, idx16, ids_f32, plan


# ----------------------------------------------------------------------------
# Device kernel
# ----------------------------------------------------------------------------

def build(nc, tc, cfg: Cfg, plan: Plan):
    c = cfg
    RG = [list(range(c.cores))]
    total_chunks = plan.total_chunks
    slots = total_chunks * 128

    x_sh = nc.dram_tensor("x_sh", [c.nloc, c.din], BF16, kind="ExternalInput").ap()
    w1 = nc.dram_tensor("w1", [c.din, c.dh], BF16, kind="ExternalInput").ap()
    w2 = nc.dram_tensor("w2", [c.dh, c.dh2], F32, kind="ExternalInput").ap()
    b1r = nc.dram_tensor("b1r", [128, c.dh], F32, kind="ExternalInput").ap()
    b2r = nc.dram_tensor("b2r", [128, c.dh2], F32, kind="ExternalInput").ap()
    degp = nc.dram_tensor("degp", [128, c.nt], F32, kind="ExternalInput").ap()
    idx16 = nc.dram_tensor("idx16", [128, slots // 16], I16, kind="ExternalInput").ap()
    idsf = nc.dram_tensor("idsf", [128, total_chunks], BF16, kind="ExternalInput").ap()
    out_h = nc.dram_tensor("out_h", [c.nloc, c.dh2], F32, kind="ExternalOutput").ap()
    out_ls = nc.dram_tensor("out_ls", [c.nloc, c.dh2], F32, kind="ExternalOutput").ap()

    t1_loc = nc.dram_tensor("t1_loc", [c.nloc, c.dh], BF16, kind="Internal").ap()
    t1_full = nc.dram_tensor(
        "t1_full", [c.trows, c.dh], BF16, kind="Internal", addr_space="Shared"
    ).ap()
    t2_loc = nc.dram_tensor("t2_loc", [c.nloc, c.dt2], BF16, kind="Internal").ap()
    t2_full = nc.dram_tensor(
        "t2_full", [c.trows, c.dt2], BF16, kind="Internal", addr_space="Shared"
    ).ap()

    with ExitStack() as st:
        cpool = st.enter_context(tc.tile_pool(name="consts", bufs=1))
        accp = st.enter_context(tc.tile_pool(name="acc", bufs=1))
        gp = st.enter_context(tc.tile_pool(name="gp", bufs=2))
        sp = st.enter_context(tc.tile_pool(name="sp", bufs=2))
        pp = st.enter_context(tc.tile_pool(name="pp", bufs=2))
        ppsum = st.enter_context(tc.tile_pool(name="ppsum", bufs=4, space="PSUM"))
        p0 = st.enter_context(tc.tile_pool(name="p0", bufs=3))
        p0ps = st.enter_context(tc.tile_pool(name="p0ps", bufs=2, space="PSUM"))
        p0psT = st.enter_context(tc.tile_pool(name="p0psT", bufs=2, space="PSUM"))

        # ---- constants ----
        ident = cpool.tile([128, 128], F32)
        make_identity(nc, ident)
        identb = cpool.tile([128, 128], BF16)
        make_identity(nc, identb)
        w1sb = cpool.tile([128, c.kt, c.dh], BF16)
        nc.sync.dma_start(w1sb, w1.rearrange("(o p) f -> p o f", p=128))
        w2sb = cpool.tile([128, c.dh2], F32)
        nc.sync.dma_start(w2sb, w2)
        b1sb = cpool.tile([128, c.dh], F32)
        nc.sync.dma_start(b1sb, b1r)
        b2sb = cpool.tile([128, c.dh2], F32)
        nc.sync.dma_start(b2sb, b2r)
        dinv = cpool.tile([128, c.nt], F32)
        nc.sync.dma_start(dinv, degp)
        nc.scalar.activation(dinv, dinv, AF.Sqrt)
        nc.vector.reciprocal(dinv, dinv)
        iota = cpool.tile([128, c.max_piece, 128], BF16)
        nc.gpsimd.iota(iota, pattern=[[0, c.max_piece], [1, 128]], base=0,
                       channel_multiplier=0,
                       allow_small_or_imprecise_dtypes=True)

        # ---- phase 0: T1 = dinv * (x @ W1), write local table shard ----
        with nc.named_scope("p0_mm1"):
            for t in range(c.nt):
                xt = p0.tile([128, c.din], BF16, tag="xt")
                nc.sync.dma_start(xt, x_sh[ts(t, 128), :])
                hps = p0ps.tile([128, c.dh], F32, tag="hps")
                for j in range(c.kt):
                    tps = p0psT.tile([128, 128], BF16, tag="tps")
                    nc.tensor.transpose(tps, xt[:, ts(j, 128)], identb)
                    xT = p0.tile([128, 128], BF16, tag="xT")
                    nc.vector.tensor_copy(xT, tps)
                    nc.tensor.matmul(
                        hps, lhsT=xT, rhs=w1sb[:, j, :],
                        start=(j == 0), stop=(j == c.kt - 1),
                    )
                hsb = p0.tile([128, c.dh], BF16, tag="hsb")
                nc.vector.tensor_scalar_mul(hsb, hps, dinv[:, t : t + 1])
                nc.sync.dma_start(t1_loc[ts(t, 128), :], hsb)

        with nc.named_scope("ag1"):
            nc.gpsimd.collective_compute(
                "AllGather", ALU.bypass, replica_groups=RG,
                ins=[t1_loc.opt()], outs=[t1_full.opt()],
            )

        # ---- edge aggregation: acc[:, b, :] += sum over block's chunks ----
        def edge_phase(table_full, t_loc, acc, d, dt):
            edge_phase.piece_ctr = getattr(edge_phase, "piece_ctr", 0)
            # seed with the self-loop term T[i] (sync DMA + DVE cast; keeps
            # the SWDGE lanes exclusively on queue-3 gathers)
            tv = t_loc.rearrange("(b p) f -> p b f", p=128)
            for t in range(c.nt):
                sd = p0.tile([128, dt], BF16, tag="sd")
                nc.sync.dma_start(sd, tv[:, t, :])
                nc.vector.tensor_copy(acc[:, t, :], sd[:, :d])
            bases = c.wbases
            max_sec = max(sum(sz) for sz in plan.sections)
            chunk0 = 0          # global chunk cursor
            for w, sizes in enumerate(plan.sections):
                q = plan.quotas[w]
                sec_ch = sum(sizes)
                sit = sp.tile([128, max_sec * 8], I16, tag="sit")
                nc.sync.dma_start(
                    sit[:, : sec_ch * 8],
                    idx16[:, chunk0 * 8 : (chunk0 + sec_ch) * 8],
                )
                sid = sp.tile([128, max_sec], BF16, tag="sid")
                nc.sync.dma_start(sid[:, :sec_ch], idsf[:, chunk0 : chunk0 + sec_ch])
                loc = 0
                k_in_block = 0
                b = 0
                ps = None
                for nch in sizes:
                    g = gp.tile([128, c.max_piece, dt], BF16, tag="gt")
                    qn = edge_phase.piece_ctr % 4
                    edge_phase.piece_ctr += 1
                    nc.gpsimd.dma_gather(
                        g[:, :nch, :], table_full[ds(bases[w], c.wsize), :],
                        sit[:, loc * 8 : (loc + nch) * 8],
                        num_idxs=nch * 128, num_idxs_reg=nch * 128, elem_size=dt,
                        single_packet=False, queue_num=qn,
                    )
                    stt = pp.tile([128, c.max_piece, 128], BF16, tag="stt")
                    nc.vector.tensor_tensor(
                        stt[:, :nch, :], iota[:, :nch, :],
                        sid[:, loc : loc + nch, None].to_broadcast((128, nch, 128)),
                        ALU.is_equal,
                    )
                    for j in range(nch):
                        if k_in_block == 0:
                            ps = ppsum.tile([128, d], F32, tag="ps")
                        nc.tensor.matmul(
                            ps, lhsT=stt[:, j, :], rhs=g[:, j, :d],
                            start=(k_in_block == 0), stop=(k_in_block == q - 1),
                        )
                        k_in_block += 1
                        if k_in_block == q:
                            nc.vector.tensor_tensor(
                                acc[:, b, :], acc[:, b, :], ps, ALU.add
                            )
                            b += 1
                            k_in_block = 0
                    loc += nch
                    chunk0 += nch
                assert b == c.nt and k_in_block == 0

        acc1 = accp.tile([128, c.nt, c.dh], F32)
        with nc.named_scope("edge1"):
            edge_phase(t1_full, t1_loc, acc1, c.dh, c.dh)

        # ---- g1 = dinv * relu(dinv * agg + b1), in place, batched ----
        dinv_bc1 = dinv[:, :, None].to_broadcast((128, c.nt, c.dh))
        nc.vector.tensor_tensor(acc1, acc1, dinv_bc1, ALU.mult)
        nc.vector.tensor_tensor(
            acc1, acc1, b1sb[:, None, :].to_broadcast((128, c.nt, c.dh)), ALU.add
        )
        nc.scalar.activation(acc1, acc1, AF.Relu)
        nc.vector.tensor_tensor(acc1, acc1, dinv_bc1, ALU.mult)

        # ---- phase 2: T2 = g1 @ W2 (row scaling already folded into g1) ----
        with nc.named_scope("p2_mm2"):
            for t in range(c.nt):
                tps = p0psT.tile([128, 128], F32, tag="tps")
                nc.tensor.transpose(tps, acc1[:, t, :], ident)
                gT = p0.tile([128, 128], F32, tag="xT")
                nc.vector.tensor_copy(gT, tps)
                h2ps = p0ps.tile([128, c.dh2], F32, tag="hps")
                nc.tensor.matmul(h2ps, lhsT=gT, rhs=w2sb, start=True, stop=True)
                h2sb = p0.tile([128, c.dh2], BF16, tag="h2sb")
                nc.vector.tensor_copy(h2sb, h2ps)
                nc.sync.dma_start(t2_loc[ts(t, 128), : c.dh2], h2sb)

        with nc.named_scope("ag2"):
            nc.gpsimd.collective_compute(
                "AllGather", ALU.bypass, replica_groups=RG,
                ins=[t2_loc.opt()], outs=[t2_full.opt()],
            )

        # ---- layer-2 edge aggregation ----
        acc2 = accp.tile([128, c.nt, c.dh2], F32)
        with nc.named_scope("edge2"):
            edge_phase(t2_full, t2_loc, acc2, c.dh2, c.dt2)

        # ---- h = dinv * agg2 + b2 ; log_softmax (batched) ----
        tail_scope = st.enter_context(nc.named_scope("tail"))  # noqa: F841
        ohv = out_h.rearrange("(t p) f -> p t f", p=128)
        olv = out_ls.rearrange("(t p) f -> p t f", p=128)
        nc.vector.tensor_tensor(
            acc2, acc2, dinv[:, :, None].to_broadcast((128, c.nt, c.dh2)), ALU.mult
        )
        nc.vector.tensor_tensor(
            acc2, acc2, b2sb[:, None, :].to_broadcast((128, c.nt, c.dh2)), ALU.add
        )
        nc.sync.dma_start(ohv, acc2)
        accN = acc2[:, :, : c.dout]
        mx = accp.tile([128, c.nt], F32, tag="mx")
        nc.vector.tensor_reduce(mx, accN, mybir.AxisListType.X, ALU.max)
        nc.vector.tensor_tensor(
            accN, accN, mx[:, :, None].to_broadcast((128, c.nt, c.dout)), ALU.subtract
        )
        e1 = accp.tile([128, c.nt, c.dout], F32, tag="e1")
        nc.scalar.activation(e1, accN, AF.Exp)
        se = accp.tile([128, c.nt], F32, tag="se")
        nc.vector.tensor_reduce(se, e1, mybir.AxisListType.X, ALU.add)
        ln = accp.tile([128, c.nt], F32, tag="ln")
        nc.scalar.activation(ln, se, AF.Ln)
        nc.vector.tensor_tensor(
            accN, accN, ln[:, :, None].to_broadcast((128, c.nt, c.dout)), ALU.subtract
        )
        nc.sync.dma_start(olv[:, :, : c.dout], accN)


# ----------------------------------------------------------------------------
# Host entry point
# ----------------------------------------------------------------------------

_CACHE = {}


def _get_compiled(cfg: Cfg, plan: Plan):
    key = (cfg, plan)
    if key not in _CACHE:
        nc = bacc.Bacc(
            "TRN2", target_bir_lowering=False, debug=False,
            num_devices=cfg.cores, num_swdge_queues=4,
        )
        with tile.TileContext(nc) as tc:
            build(nc, tc, cfg, plan)
        nc.compile()
        _CACHE[key] = nc
    return _CACHE[key]


def make_in_maps(cfg: Cfg, x, W1, b1, W2, b2, deg_pt, idx16, ids_f32):
    import ml_dtypes

    c = cfg
    x = np.asarray(x, np.float32)
    w2p = np.zeros((c.dh, c.dh2), np.float32)
    w2p[:, : c.dout] = np.asarray(W2, np.float32)
    b1rep = np.tile(np.asarray(b1, np.float32)[None, :], (128, 1))
    b2p = np.zeros(c.dh2, np.float32)
    b2p[: c.dout] = np.asarray(b2, np.float32)
    b2rep = np.tile(b2p[None, :], (128, 1))
    w1c = np.ascontiguousarray(
        np.asarray(W1, np.float32).astype(ml_dtypes.bfloat16)
    )

    in_maps = []
    for ci in range(c.cores):
        xs = np.zeros((c.nloc, c.din), ml_dtypes.bfloat16)
        xs[: c.nsh] = x[ci * c.nsh : (ci + 1) * c.nsh].astype(ml_dtypes.bfloat16)
        in_maps.append({
            "x_sh": xs,
            "w1": w1c,
            "w2": w2p,
            "b1r": b1rep,
            "b2r": b2rep,
            "degp": np.ascontiguousarray(deg_pt[ci]),
            "idx16": np.ascontiguousarray(idx16[ci]),
            "idsf": np.ascontiguousarray(ids_f32[ci].astype(ml_dtypes.bfloat16)),
        })
    return in_maps


def _ensure_ntff_hook():
    """Install the axon NTFF profile hook if the image's antenv lacks it."""
    import types

    try:
        from antenv.axon_hooks import get_axon_ntff_profile_hook  # noqa: F401
        return
    except ImportError:
        pass
    import antenv

    m = types.ModuleType("antenv.axon_hooks")
    m._hook = None
    m.set_axon_ntff_profile_hook = lambda h: setattr(m, "_hook", h)
    m.get_axon_ntff_profile_hook = lambda: m._hook
    sys.modules["antenv.axon_hooks"] = m
    antenv.axon_hooks = m
    try:
        from trn_agent_boot.trn_boot import _ntff_profile_via_ctypes

        h = _ntff_profile_via_ctypes("/opt/axon/libaxon_pjrt.so")
        if h is not None:
            m._hook = h
    except Exception as e:
        print(f"ntff hook install failed: {e}")

    from concourse import bass_utils as bu

    bu.upload_artifacts = lambda tmpdir: tmpdir


def run(cfg: Cfg, inputs: dict, trace: bool = False):
    if trace:
        _ensure_ntff_hook()
    deg_pt, idx16, ids_f32, plan = preprocess(cfg, inputs["edge_index"])
    nc = _get_compiled(cfg, plan)
    in_maps = make_in_maps(
        cfg, inputs["x"], inputs["W1"], inputs["b1"], inputs["W2"], inputs["b2"],
        deg_pt, idx16, ids_f32,
    )
    res = run_bass_kernel_spmd(
        nc, in_maps, core_ids=list(range(cfg.cores)), trace=trace
    )
    c = cfg
    h = np.concatenate(
        [res.results[ci]["out_h"][: c.nsh, : c.dout] for ci in range(c.cores)], axis=0
    )
    ls = np.concatenate(
        [res.results[ci]["out_ls"][: c.nsh, : c.dout] for ci in range(c.cores)], axis=0
    )
    return (h, ls), res


def kernel(**inputs):
    (h, ls), _ = run(Cfg(), inputs)
    return h, ls



# revision 10
# speedup vs baseline: 1.1959x; 1.0366x over previous
"""Trainium2 Bass kernel for a 2-layer GCN (nn_EvenLamerGCN).

reference semantics (PyG GCNConv x2, eval mode):
    deg[i]  = 1 + indeg(i)                (self-loops added)
    dinv    = deg ** -0.5
    h  = relu(A_hat @ (x @ W1) + b1),  A_hat = D^-1/2 (A + I) D^-1/2
    o  = A_hat @ (h @ W2) + b2
    return o, log_softmax(o, axis=1)

Distribution: nodes sharded over 8 NeuronCores (12500/core, padded to
12544), edges partitioned by destination core.  The per-edge norm is
folded into per-node row scalings:
    out = dinv * ( sum_{e: dst=i} T[src_e] + T[i] ),   T = dinv * (x @ W)

Per layer on each core:
  1. dense matmul -> row-scaled table shard T_c, AllGather -> full T
  2. per-edge dma_gather of T[src] rows (128-row chunks, int16 idx against
     static table windows)
  3. segment-sum via one-hot matmul: S^T[e, d] = (dst_local[e] % 128 == d)
     built on DVE from shipped per-slot ids, then PSUM-accumulated
     out_block += S^T.T @ gathered, drained into a [128, 98, d] SBUF
     accumulator seeded with the self-loop term.
Edges are laid out per (dst-block, src-window) cell with a uniform
chunk quota so the instruction stream is identical on all 8 cores
(SPMD, one NEFF); all per-core variation lives in input data.
"""

import sys

for _p in ("/opt/trn_rl_repo", "/root/.axon_site/_ro/trn_rl_repo"):
    if _p not in sys.path:
        sys.path.insert(0, _p)

from contextlib import ExitStack
from dataclasses import dataclass

import numpy as np

import concourse.bass as bass
import concourse.mybir as mybir
import concourse.tile as tile
from concourse import bacc
from concourse.bass import ds, ts
from concourse.bass_utils import run_bass_kernel_spmd
from concourse.masks import make_identity

F32 = mybir.dt.float32
BF16 = mybir.dt.bfloat16
I16 = mybir.dt.int16
AF = mybir.ActivationFunctionType
ALU = mybir.AluOpType


@dataclass(frozen=True)
class Cfg:
    n: int = 100000          # nodes
    din: int = 512           # input features
    dh: int = 128            # hidden features
    dout: int = 40           # output features
    cores: int = 8
    wsize: int = 32768       # int16 gather window (rows)
    max_piece: int = 16      # chunks per gather instruction
    dma_scratch: int = 32768  # SWDGE descriptor carveout (bytes; ndesc = /16)

    @property
    def nsh(self):           # real nodes per core
        return self.n // self.cores

    @property
    def nloc(self):          # padded nodes per core (multiple of 128)
        return ((self.nsh + 127) // 128) * 128

    @property
    def nt(self):            # 128-node dst blocks per core
        return self.nloc // 128

    @property
    def trows(self):         # rows in the gathered tables
        return self.cores * self.nloc

    @property
    def dh2(self):           # padded output width
        return max(64, ((self.dout + 63) // 64) * 64)

    @property
    def dt2(self):           # layer-2 bf16 table row width (256B rows)
        return max(128, self.dh2)

    @property
    def kt(self):            # k-tiles in the first matmul
        return self.din // 128

    @property
    def nwin(self):          # number of static src windows
        return max(1, -(-self.trows // self.wsize))

    @property
    def wbases(self):
        return [min(w * self.wsize, self.trows - self.wsize)
                for w in range(self.nwin)]


@dataclass(frozen=True)
class Plan:
    quotas: tuple          # chunks per (window) cell, per dst block
    sections: tuple        # per window: list of piece sizes (in chunks)

    @property
    def chunks_per_block(self):
        return sum(self.quotas)

    @property
    def total_chunks(self):
        return sum(sum(s) for s in self.sections)


# ----------------------------------------------------------------------------
# CPU-side preprocessing
# ----------------------------------------------------------------------------

def preprocess(cfg: Cfg, edge_index: np.ndarray):
    c = cfg
    src = np.asarray(edge_index[0], dtype=np.int64)
    dst = np.asarray(edge_index[1], dtype=np.int64)

    deg = np.bincount(dst, minlength=c.n).astype(np.float32) + 1.0
    deg_pt = np.ones((c.cores, 128, c.nt), np.float32)
    for ci in range(c.cores):
        dl = np.ones(c.nloc, np.float32)
        dl[: c.nsh] = deg[ci * c.nsh : (ci + 1) * c.nsh]
        deg_pt[ci] = dl.reshape(c.nt, 128).T

    row_of = lambda i: (i // c.nsh) * c.nloc + (i % c.nsh)
    r_all = row_of(src)
    w_all = np.minimum(r_all // c.wsize, c.nwin - 1)
    core_all = dst // c.nsh
    dloc_all = dst - core_all * c.nsh
    b_all = dloc_all // 128
    id_all = dloc_all % 128

    # count edges per (core, block, window) -> uniform chunk quotas
    cell_key = (core_all * c.nt + b_all) * c.nwin + w_all
    counts = np.bincount(cell_key, minlength=c.cores * c.nt * c.nwin)
    counts = counts.reshape(c.cores, c.nt, c.nwin)
    quotas = tuple(int(-(-counts[:, :, w].max() // 128)) for w in range(c.nwin))

    # piece sizes (chunks) per window section
    sections = []
    for w in range(c.nwin):
        sec = c.nt * quotas[w]
        sizes = []
        while sec > 0:
            sizes.append(min(c.max_piece, sec))
            sec -= sizes[-1]
        sections.append(tuple(sizes))
    plan = Plan(quotas=quotas, sections=tuple(sections))

    total_chunks = plan.total_chunks
    slots = total_chunks * 128
    bases = c.wbases

    idx16 = np.zeros((c.cores, 128, slots // 16), np.int16)
    ids_f32 = np.empty((c.cores, 128, total_chunks), np.float32)

    order = np.lexsort((r_all, w_all, b_all, core_all))
    so_r, so_w, so_b, so_core, so_id = (
        r_all[order], w_all[order], b_all[order], core_all[order], id_all[order]
    )
    core_starts = np.searchsorted(so_core, np.arange(c.cores + 1))

    for ci in range(c.cores):
        lo, hi = core_starts[ci], core_starts[ci + 1]
        rr, ii = so_r[lo:hi], so_id[lo:hi]
        rel = np.zeros(slots, np.int64)      # window-relative gather rows
        ids = np.full(slots, -1.0, np.float32)
        # slot offset of window section w
        sec_off = np.cumsum([0] + [c.nt * q * 128 for q in quotas])
        pos = 0
        # sorted order within a core is (b, w, r); cells land at
        # sec_off[w] + b * quotas[w] * 128
        for b in range(c.nt):
            for w in range(c.nwin):
                cnt = counts[ci, b, w]
                if cnt:
                    off = sec_off[w] + b * quotas[w] * 128
                    rel[off : off + cnt] = rr[pos : pos + cnt] - bases[w]
                    ids[off : off + cnt] = ii[pos : pos + cnt]
                    pos += cnt
        assert pos == hi - lo
        assert rel.min() >= 0 and rel.max() < c.wsize

        v = rel.reshape(-1, 16)              # slot i at [i%16, i//16]
        wrapped = np.ascontiguousarray(v.T)  # [16, slots/16]
        idx16[ci] = np.tile(wrapped, (8, 1)).astype(np.int16)
        ids_f32[ci] = ids.reshape(total_chunks, 128).T

    return deg_pt, idx16, ids_f32, plan


# ----------------------------------------------------------------------------
# Device kernel
# ----------------------------------------------------------------------------

def build(nc, tc, cfg: Cfg, plan: Plan):
    c = cfg
    RG = [list(range(c.cores))]
    total_chunks = plan.total_chunks
    slots = total_chunks * 128

    x_sh = nc.dram_tensor("x_sh", [c.nloc, c.din], BF16, kind="ExternalInput").ap()
    w1 = nc.dram_tensor("w1", [c.din, c.dh], BF16, kind="ExternalInput").ap()
    w2 = nc.dram_tensor("w2", [c.dh, c.dh2], F32, kind="ExternalInput").ap()
    b1r = nc.dram_tensor("b1r", [128, c.dh], F32, kind="ExternalInput").ap()
    b2r = nc.dram_tensor("b2r", [128, c.dh2], F32, kind="ExternalInput").ap()
    degp = nc.dram_tensor("degp", [128, c.nt], F32, kind="ExternalInput").ap()
    idx16 = nc.dram_tensor("idx16", [128, slots // 16], I16, kind="ExternalInput").ap()
    idsf = nc.dram_tensor("idsf", [128, total_chunks], BF16, kind="ExternalInput").ap()
    out_h = nc.dram_tensor("out_h", [c.nloc, c.dh2], F32, kind="ExternalOutput").ap()
    out_ls = nc.dram_tensor("out_ls", [c.nloc, c.dh2], F32, kind="ExternalOutput").ap()

    t1_loc = nc.dram_tensor("t1_loc", [c.nloc, c.dh], BF16, kind="Internal").ap()
    t1_full = nc.dram_tensor(
        "t1_full", [c.trows, c.dh], BF16, kind="Internal", addr_space="Shared"
    ).ap()
    t2_loc = nc.dram_tensor("t2_loc", [c.nloc, c.dt2], BF16, kind="Internal").ap()
    t2_full = nc.dram_tensor(
        "t2_full", [c.trows, c.dt2], BF16, kind="Internal", addr_space="Shared"
    ).ap()

    with ExitStack() as st:
        cpool = st.enter_context(tc.tile_pool(name="consts", bufs=1))
        accp = st.enter_context(tc.tile_pool(name="acc", bufs=1))
        gp = st.enter_context(tc.tile_pool(name="gp", bufs=2))
        sp = st.enter_context(tc.tile_pool(name="sp", bufs=2))
        pp = st.enter_context(tc.tile_pool(name="pp", bufs=2))
        ppsum = st.enter_context(tc.tile_pool(name="ppsum", bufs=4, space="PSUM"))
        p0 = st.enter_context(tc.tile_pool(name="p0", bufs=3))
        p0ps = st.enter_context(tc.tile_pool(name="p0ps", bufs=2, space="PSUM"))
        p0psT = st.enter_context(tc.tile_pool(name="p0psT", bufs=2, space="PSUM"))

        # ---- constants ----
        ident = cpool.tile([128, 128], F32)
        make_identity(nc, ident)
        identb = cpool.tile([128, 128], BF16)
        make_identity(nc, identb)
        w1sb = cpool.tile([128, c.kt, c.dh], BF16)
        nc.sync.dma_start(w1sb, w1.rearrange("(o p) f -> p o f", p=128))
        w2sb = cpool.tile([128, c.dh2], F32)
        nc.sync.dma_start(w2sb, w2)
        b1sb = cpool.tile([128, c.dh], F32)
        nc.sync.dma_start(b1sb, b1r)
        b2sb = cpool.tile([128, c.dh2], F32)
        nc.sync.dma_start(b2sb, b2r)
        dinv = cpool.tile([128, c.nt], F32)
        nc.sync.dma_start(dinv, degp)
        nc.scalar.activation(dinv, dinv, AF.Sqrt)
        nc.vector.reciprocal(dinv, dinv)
        iota = cpool.tile([128, c.max_piece, 128], BF16)
        nc.gpsimd.iota(iota, pattern=[[0, c.max_piece], [1, 128]], base=0,
                       channel_multiplier=0,
                       allow_small_or_imprecise_dtypes=True)

        # ---- phase 0: T1 = dinv * (x @ W1), write local table shard ----
        with nc.named_scope("p0_mm1"):
            for t in range(c.nt):
                xt = p0.tile([128, c.din], BF16, tag="xt")
                nc.sync.dma_start(xt, x_sh[ts(t, 128), :])
                hps = p0ps.tile([128, c.dh], F32, tag="hps")
                for j in range(c.kt):
                    tps = p0psT.tile([128, 128], BF16, tag="tps")
                    nc.tensor.transpose(tps, xt[:, ts(j, 128)], identb)
                    xT = p0.tile([128, 128], BF16, tag="xT")
                    nc.vector.tensor_copy(xT, tps)
                    nc.tensor.matmul(
                        hps, lhsT=xT, rhs=w1sb[:, j, :],
                        start=(j == 0), stop=(j == c.kt - 1),
                    )
                hsb = p0.tile([128, c.dh], BF16, tag="hsb")
                nc.vector.tensor_scalar_mul(hsb, hps, dinv[:, t : t + 1])
                nc.sync.dma_start(t1_loc[ts(t, 128), :], hsb)

        with nc.named_scope("ag1"):
            nc.gpsimd.collective_compute(
                "AllGather", ALU.bypass, replica_groups=RG,
                ins=[t1_loc.opt()], outs=[t1_full.opt()],
            )

        # ---- edge aggregation: acc[:, b, :] += sum over block's chunks ----
        def edge_phase(table_full, t_loc, acc, d, dt):
            edge_phase.piece_ctr = getattr(edge_phase, "piece_ctr", 0)
            # seed with the self-loop term T[i] (sync DMA + DVE cast; keeps
            # the SWDGE lanes exclusively on queue-3 gathers)
            tv = t_loc.rearrange("(b p) f -> p b f", p=128)
            for t in range(c.nt):
                sd = p0.tile([128, dt], BF16, tag="sd")
                nc.sync.dma_start(sd, tv[:, t, :])
                nc.vector.tensor_copy(acc[:, t, :], sd[:, :d])
            bases = c.wbases
            max_sec = max(sum(sz) for sz in plan.sections)
            chunk0 = 0          # global chunk cursor
            for w, sizes in enumerate(plan.sections):
                q = plan.quotas[w]
                sec_ch = sum(sizes)
                sit = sp.tile([128, max_sec * 8], I16, tag="sit")
                nc.sync.dma_start(
                    sit[:, : sec_ch * 8],
                    idx16[:, chunk0 * 8 : (chunk0 + sec_ch) * 8],
                )
                sid = sp.tile([128, max_sec], BF16, tag="sid")
                nc.sync.dma_start(sid[:, :sec_ch], idsf[:, chunk0 : chunk0 + sec_ch])
                loc = 0
                k_in_block = 0
                b = 0
                ps = None
                for nch in sizes:
                    g = gp.tile([128, c.max_piece, dt], BF16, tag="gt")
                    qn = edge_phase.piece_ctr % 4
                    edge_phase.piece_ctr += 1
                    nc.gpsimd.dma_gather(
                        g[:, :nch, :], table_full[ds(bases[w], c.wsize), :],
                        sit[:, loc * 8 : (loc + nch) * 8],
                        num_idxs=nch * 128, num_idxs_reg=nch * 128, elem_size=dt,
                        single_packet=False, queue_num=qn,
                    )
                    stt = pp.tile([128, c.max_piece, 128], BF16, tag="stt")
                    nc.vector.tensor_tensor(
                        stt[:, :nch, :], iota[:, :nch, :],
                        sid[:, loc : loc + nch, None].to_broadcast((128, nch, 128)),
                        ALU.is_equal,
                    )
                    for j in range(nch):
                        if k_in_block == 0:
                            ps = ppsum.tile([128, d], F32, tag="ps")
                        nc.tensor.matmul(
                            ps, lhsT=stt[:, j, :], rhs=g[:, j, :d],
                            start=(k_in_block == 0), stop=(k_in_block == q - 1),
                        )
                        k_in_block += 1
                        if k_in_block == q:
                            nc.vector.tensor_tensor(
                                acc[:, b, :], acc[:, b, :], ps, ALU.add
                            )
                            b += 1
                            k_in_block = 0
                    loc += nch
                    chunk0 += nch
                assert b == c.nt and k_in_block == 0

        acc1 = accp.tile([128, c.nt, c.dh], F32)
        with nc.named_scope("edge1"):
            edge_phase(t1_full, t1_loc, acc1, c.dh, c.dh)

        # ---- g1 = dinv * relu(dinv * agg + b1), in place, batched ----
        dinv_bc1 = dinv[:, :, None].to_broadcast((128, c.nt, c.dh))
        nc.vector.tensor_tensor(acc1, acc1, dinv_bc1, ALU.mult)
        nc.vector.tensor_tensor(
            acc1, acc1, b1sb[:, None, :].to_broadcast((128, c.nt, c.dh)), ALU.add
        )
        nc.scalar.activation(acc1, acc1, AF.Relu)
        nc.vector.tensor_tensor(acc1, acc1, dinv_bc1, ALU.mult)

        # ---- phase 2: T2 = g1 @ W2 (row scaling already folded into g1) ----
        with nc.named_scope("p2_mm2"):
            for t in range(c.nt):
                tps = p0psT.tile([128, 128], F32, tag="tps")
                nc.tensor.transpose(tps, acc1[:, t, :], ident)
                gT = p0.tile([128, 128], F32, tag="xT")
                nc.vector.tensor_copy(gT, tps)
                h2ps = p0ps.tile([128, c.dh2], F32, tag="hps")
                nc.tensor.matmul(h2ps, lhsT=gT, rhs=w2sb, start=True, stop=True)
                h2sb = p0.tile([128, c.dh2], BF16, tag="h2sb")
                nc.vector.tensor_copy(h2sb, h2ps)
                nc.sync.dma_start(t2_loc[ts(t, 128), : c.dh2], h2sb)

        with nc.named_scope("ag2"):
            nc.gpsimd.collective_compute(
                "AllGather", ALU.bypass, replica_groups=RG,
                ins=[t2_loc.opt()], outs=[t2_full.opt()],
            )

        # ---- layer-2 edge aggregation ----
        acc2 = accp.tile([128, c.nt, c.dh2], F32)
        with nc.named_scope("edge2"):
            edge_phase(t2_full, t2_loc, acc2, c.dh2, c.dt2)

        # ---- h = dinv * agg2 + b2 ; log_softmax (batched) ----
        tail_scope = st.enter_context(nc.named_scope("tail"))  # noqa: F841
        ohv = out_h.rearrange("(t p) f -> p t f", p=128)
        olv = out_ls.rearrange("(t p) f -> p t f", p=128)
        nc.vector.tensor_tensor(
            acc2, acc2, dinv[:, :, None].to_broadcast((128, c.nt, c.dh2)), ALU.mult
        )
        nc.vector.tensor_tensor(
            acc2, acc2, b2sb[:, None, :].to_broadcast((128, c.nt, c.dh2)), ALU.add
        )
        nc.sync.dma_start(ohv, acc2)
        accN = acc2[:, :, : c.dout]
        mx = accp.tile([128, c.nt], F32, tag="mx")
        nc.vector.tensor_reduce(mx, accN, mybir.AxisListType.X, ALU.max)
        nc.vector.tensor_tensor(
            accN, accN, mx[:, :, None].to_broadcast((128, c.nt, c.dout)), ALU.subtract
        )
        e1 = accp.tile([128, c.nt, c.dout], F32, tag="e1")
        nc.scalar.activation(e1, accN, AF.Exp)
        se = accp.tile([128, c.nt], F32, tag="se")
        nc.vector.tensor_reduce(se, e1, mybir.AxisListType.X, ALU.add)
        ln = accp.tile([128, c.nt], F32, tag="ln")
        nc.scalar.activation(ln, se, AF.Ln)
        nc.vector.tensor_tensor(
            accN, accN, ln[:, :, None].to_broadcast((128, c.nt, c.dout)), ALU.subtract
        )
        nc.sync.dma_start(olv[:, :, : c.dout], accN)


# ----------------------------------------------------------------------------
# Host entry point
# ----------------------------------------------------------------------------

_CACHE = {}


def _get_compiled(cfg: Cfg, plan: Plan):
    key = (cfg, plan)
    if key not in _CACHE:
        nc = bacc.Bacc(
            "TRN2", target_bir_lowering=False, debug=False,
            num_devices=cfg.cores, num_swdge_queues=4,
            dynamic_dma_scratch_size=cfg.dma_scratch,
        )
        with tile.TileContext(nc) as tc:
            build(nc, tc, cfg, plan)
        nc.compile()
        _CACHE[key] = nc
    return _CACHE[key]


def make_in_maps(cfg: Cfg, x, W1, b1, W2, b2, deg_pt, idx16, ids_f32):
    import ml_dtypes

    c = cfg
    x = np.asarray(x, np.float32)
    w2p = np.zeros((c.dh, c.dh2), np.float32)
    w2p[:, : c.dout] = np.asarray(W2, np.float32)
    b1rep = np.tile(np.asarray(b1, np.float32)[None, :], (128, 1))
    b2p = np.zeros(c.dh2, np.float32)
    b2p[: c.dout] = np.asarray(b2, np.float32)
    b2rep = np.tile(b2p[None, :], (128, 1))
    w1c = np.ascontiguousarray(
        np.asarray(W1, np.float32).astype(ml_dtypes.bfloat16)
    )

    in_maps = []
    for ci in range(c.cores):
        xs = np.zeros((c.nloc, c.din), ml_dtypes.bfloat16)
        xs[: c.nsh] = x[ci * c.nsh : (ci + 1) * c.nsh].astype(ml_dtypes.bfloat16)
        in_maps.append({
            "x_sh": xs,
            "w1": w1c,
            "w2": w2p,
            "b1r": b1rep,
            "b2r": b2rep,
            "degp": np.ascontiguousarray(deg_pt[ci]),
            "idx16": np.ascontiguousarray(idx16[ci]),
            "idsf": np.ascontiguousarray(ids_f32[ci].astype(ml_dtypes.bfloat16)),
        })
    return in_maps


def _ensure_ntff_hook():
    """Install the axon NTFF profile hook if the image's antenv lacks it."""
    import types

    try:
        from antenv.axon_hooks import get_axon_ntff_profile_hook  # noqa: F401
        return
    except ImportError:
        pass
    import antenv

    m = types.ModuleType("antenv.axon_hooks")
    m._hook = None
    m.set_axon_ntff_profile_hook = lambda h: setattr(m, "_hook", h)
    m.get_axon_ntff_profile_hook = lambda: m._hook
    sys.modules["antenv.axon_hooks"] = m
    antenv.axon_hooks = m
    try:
        from trn_agent_boot.trn_boot import _ntff_profile_via_ctypes

        h = _ntff_profile_via_ctypes("/opt/axon/libaxon_pjrt.so")
        if h is not None:
            m._hook = h
    except Exception as e:
        print(f"ntff hook install failed: {e}")

    from concourse import bass_utils as bu

    bu.upload_artifacts = lambda tmpdir: tmpdir


def run(cfg: Cfg, inputs: dict, trace: bool = False):
    if trace:
        _ensure_ntff_hook()
    deg_pt, idx16, ids_f32, plan = preprocess(cfg, inputs["edge_index"])
    nc = _get_compiled(cfg, plan)
    in_maps = make_in_maps(
        cfg, inputs["x"], inputs["W1"], inputs["b1"], inputs["W2"], inputs["b2"],
        deg_pt, idx16, ids_f32,
    )
    res = run_bass_kernel_spmd(
        nc, in_maps, core_ids=list(range(cfg.cores)), trace=trace
    )
    c = cfg
    h = np.concatenate(
        [res.results[ci]["out_h"][: c.nsh, : c.dout] for ci in range(c.cores)], axis=0
    )
    ls = np.concatenate(
        [res.results[ci]["out_ls"][: c.nsh, : c.dout] for ci in range(c.cores)], axis=0
    )
    return (h, ls), res


def kernel(**inputs):
    (h, ls), _ = run(Cfg(), inputs)
    return h, ls

